# revision 49
# baseline (speedup 1.0000x reference)
"""Trainium2 Bass kernel for a pre-norm transformer decoder layer.

Full inputs in, full output out. 8-way data-parallel over tokens
(batch 2 x 4 query-slices of 512). Each core redundantly computes K/V
for its batch's full 2048-token sequence, interleaved with the
attention loop so the K/V matmuls hide under the ACT-bound softmax exp.

Shapes: x (2, 2048, 1024), 16 heads, dk=64, d_ff=2048, eps=1e-5.
"""
import threading

import numpy as np
import ml_dtypes

import concourse.mybir as mybir
import concourse.tile as tile
from concourse import bacc
from concourse.bass_utils import run_bass_kernel_spmd
from contextlib import ExitStack

F32 = mybir.dt.float32
BF16 = mybir.dt.bfloat16
AF = mybir.ActivationFunctionType
OP = mybir.AluOpType

B, S, D = 2, 2048, 1024
H, DK, FF = 16, 64, 2048
EPS = 1e-5
NCORES = 8
SQ = S * B // NCORES          # 512 own query tokens per core
ND = D // 128                 # 8 feature tiles
NT = S // 128                 # 16 sequence tiles
NTQ = SQ // 128               # 4 own-token tiles
NF = FF // 128                # 16 ff tiles
NHP = H // 2                  # 8 head pairs
NG = 4                        # head-quarter groups (2 head pairs each)

_BF = ml_dtypes.bfloat16


def _build_nc():
    nc = bacc.Bacc("TRN2", target_bir_lowering=False, debug=False,
                   num_devices=NCORES)

    x = nc.dram_tensor("x", [S, D], BF16, kind="ExternalInput").ap()
    wqt = nc.dram_tensor("wqt", [D, D], BF16, kind="ExternalInput").ap()
    wkt = nc.dram_tensor("wkt", [D, D], BF16, kind="ExternalInput").ap()
    wvt = nc.dram_tensor("wvt", [D, D], BF16, kind="ExternalInput").ap()
    wot = nc.dram_tensor("wot", [D, D], BF16, kind="ExternalInput").ap()
    w1t = nc.dram_tensor("w1t", [D, FF], BF16, kind="ExternalInput").ap()
    w2t = nc.dram_tensor("w2t", [FF, D], BF16, kind="ExternalInput").ap()
    bqd = nc.dram_tensor("bq", [128, ND], F32, kind="ExternalInput").ap()
    bkd = nc.dram_tensor("bk", [128, ND], F32, kind="ExternalInput").ap()
    bvd = nc.dram_tensor("bv", [1, D], BF16, kind="ExternalInput").ap()
    bod = nc.dram_tensor("bo", [1, D], BF16, kind="ExternalInput").ap()
    b1d = nc.dram_tensor("b1", [128, NF], F32, kind="ExternalInput").ap()
    b2d = nc.dram_tensor("b2", [1, D], BF16, kind="ExternalInput").ap()
    outd = nc.dram_tensor("out", [SQ, D], BF16, kind="ExternalOutput").ap()

    with tile.TileContext(nc) as tc, ExitStack() as ctx:
        # ---- whole-kernel pools ----
        const = ctx.enter_context(tc.tile_pool(name="const", bufs=1))
        xP = ctx.enter_context(tc.tile_pool(name="xP", bufs=1))
        qfmP = ctx.enter_context(tc.tile_pool(name="qfmP", bufs=1))
        kP = ctx.enter_context(tc.tile_pool(name="kP", bufs=4))
        vP = ctx.enter_context(tc.tile_pool(name="vP", bufs=32))
        ofmP = ctx.enter_context(tc.tile_pool(name="ofmP", bufs=1))
        lns = ctx.enter_context(tc.tile_pool(name="lns", bufs=8))
        stp = ctx.enter_context(tc.tile_pool(name="stp", bufs=3))
        xresP = ctx.enter_context(tc.tile_pool(name="xresP", bufs=1))

        # ---- constants ----
        eps_sb = const.tile([128, 1], F32, tag="eps")
        nc.vector.memset(eps_sb, EPS)
        ones_sb = const.tile([1, 128], BF16, tag="ones")
        nc.vector.memset(ones_sb, 1.0)
        onesc = const.tile([128, 1], BF16, tag="onesc")
        nc.vector.memset(onesc, 1.0)
        bq_sb = const.tile([128, ND], F32, tag="bq")
        bk_sb = const.tile([128, ND], F32, tag="bk")
        bo_bf = const.tile([1, D], BF16, tag="bo")
        b1_sb = const.tile([128, NF], F32, tag="b1")
        b2_bf = const.tile([1, D], BF16, tag="b2")
        bv_bc = const.tile([128, D], BF16, tag="bvb")

        # ---- tile allocation (pool stack: LIFO closes) ----
        def walloc(pool, dname, blocks, cols):
            return pool.tile([128, blocks, cols], BF16, tag=dname, name=dname)

        def wemit(w, dram, q):
            q.dma_start(out=w, in_=dram.rearrange("(d p) c -> p d c", p=128))
            return w

        def wload(pool, dname, dram, blocks, q=nc.gpsimd):
            return wemit(walloc(pool, dname, blocks, dram.shape[1]), dram, q)

        wkP = ExitStack()
        wk_all = walloc(wkP.enter_context(tc.tile_pool(name="wkP", bufs=1)),
                        "wk", ND, D)
        wvP = ExitStack()
        wv_all = walloc(wvP.enter_context(tc.tile_pool(name="wvP", bufs=1)),
                        "wv", ND, D)
        wo_all = [None]   # loaded at g==3, after zq/wk/wv free
        w1_all = [None]
        w2_all = [None]

        # zq[i][p, j, t]: feature-major z quadrant i; zq[i][:, j, t] holds
        # feature j*128+p of token i*512+t
        zqStack = ExitStack()
        zqP = zqStack.enter_context(tc.tile_pool(name="zqP", bufs=1))
        zq = [zqP.tile([128, ND, 512], BF16, tag=f"zq{i}", name=f"zq{i}")
              for i in range(4)]

        wqP = ExitStack()
        wq_all = walloc(wqP.enter_context(tc.tile_pool(name="wqP", bufs=1)),
                        "wq", ND, D)

        # xq[i][p, t, c] = x[i*512 + t*128 + p, c]; quadrant 0 (own tokens)
        # stays resident for the residual adds, 1-3 free after LN1
        xPr = ExitStack()
        xPr_pool = xPr.enter_context(tc.tile_pool(name="xPr", bufs=2))
        xq = [(xP if i == 0 else xPr_pool).tile(
                  [128, 4, D], BF16, tag=("xq0" if i == 0 else "xqr"),
                  name=f"xq{i}") for i in range(4)]

        # ---- DMA emission, bandwidth-prioritized ----
        # Each HW-DGE queue moves ~105 GB/s serially, so split x across the
        # sync and scalar DGEs; weights go through gpsimd's software DGE
        # (that engine is idle until the first attention normalization).
        def xemit(i, q):
            q.dma_start(
                out=xq[i],
                in_=x[i * 512:(i + 1) * 512, :].rearrange(
                    "(t p) c -> p t c", p=128))
        xemit(0, nc.sync)
        xemit(1, nc.scalar)
        nc.sync.dma_start(out=bq_sb, in_=bqd)
        nc.sync.dma_start(out=bk_sb, in_=bkd)
        nc.sync.dma_start(out=bv_bc[0:1, :], in_=bvd)
        nc.sync.dma_start(out=bo_bf, in_=bod)
        nc.sync.dma_start(out=b1_sb, in_=b1d)
        nc.sync.dma_start(out=b2_bf, in_=b2d)
        # wq first on gpsimd's software DGE (idle engine; Q needs it first)
        wemit(wq_all, wqt, nc.gpsimd)
        nc.gpsimd.partition_broadcast(bv_bc, bv_bc[0:1, :])
        wemit(wk_all, wkt, nc.gpsimd)
        wemit(wv_all, wvt, nc.gpsimd)

        # ---- LN1: stats on DVE, apply on ACT, transpose sync/scalar ----

        def ln_tile(x_src, z_dst_q, z_dst_off, tag, tq=None):
            st = lns.tile([128, 2, 6], F32, tag="st")
            nc.vector.bn_stats(st[:, 0, :], x_src[:, 0:512])
            nc.vector.bn_stats(st[:, 1, :], x_src[:, 512:1024])
            mv = lns.tile([128, 2], F32, tag="mv")
            nc.vector.bn_aggr(mv, st)
            sq = lns.tile([128, 1], F32, tag="sq")
            nc.scalar.activation(sq, mv[:, 1:2], AF.Sqrt, bias=eps_sb[:, 0:1],
                                 scale=1.0)
            rstd = lns.tile([128, 1], F32, tag="rstd")
            nc.vector.reciprocal_approx_fast(rstd, sq)
            nmr = lns.tile([128, 1], F32, tag="nmr")
            nc.vector.tensor_scalar(nmr, mv[:, 0:1], rstd, -1.0,
                                    op0=OP.mult, op1=OP.mult)
            z_tm = lns.tile([128, D], BF16, tag=tag, bufs=3)
            nc.scalar.activation(z_tm, x_src, AF.Identity, bias=nmr,
                                 scale=rstd)
            (tq or nc.sync).dma_start_transpose(
                z_dst_q[:, :, z_dst_off:z_dst_off + 128], z_tm)

        # quadrant 0's LN + transposes go ahead of the x2/x3 kicks so the
        # DGEs aren't busy with x when zq[0] (which gates Q) is ready
        for tt in range(4):
            ln_tile(xq[0][:, tt, :], zq[0], tt * 128, "ztm", tq=nc.sync)
        with tc.tile_wait_until(0.024):
            xemit(2, nc.sync)
            xemit(3, nc.scalar)
        for i in range(1, 4):
            for tt in range(4):
                ln_tile(xq[i][:, tt, :], zq[i], tt * 128, "ztm",
                        tq=(nc.sync if i % 2 == 0 else nc.scalar))
        xPr.close()

        # ---- PSUM pools (attention region: 1+1+4+2 = 8 banks) ----
        attPs = ExitStack()
        psK = attPs.enter_context(tc.tile_pool(name="psK", bufs=1,
                                               space="PSUM"))
        psV = attPs.enter_context(tc.tile_pool(name="psV", bufs=1,
                                               space="PSUM"))
        pgp = attPs.enter_context(tc.tile_pool(name="pgp", bufs=2,
                                               space="PSUM"))
        ppvp = attPs.enter_context(tc.tile_pool(name="ppvp", bufs=2,
                                                space="PSUM"))

        # ---- Q: own 512 tokens, feature-major (uses psV's slot early) ----
        q_fm = [qfmP.tile([128, SQ], BF16, tag=f"q{d}", name=f"q{d}")
                for d in range(ND)]
        for j in range(ND):
            pq = pgp.tile([128, 2, 512], F32, tag="pg")
            for d in range(ND):
                nc.tensor.matmul(pq[:, 0, :],
                                 wq_all[:, d, j * 128:(j + 1) * 128],
                                 zq[0][:, d, :],
                                 start=(d == 0), stop=(d == ND - 1))
            nc.vector.tensor_scalar(q_fm[j], pq[:, 0, :], bq_sb[:, j:j + 1],
                                    None, op0=OP.add)
        wqP.close()

        # ---- K/V compute, emitted as "filler" generators per quarter ----
        # k_fm tiles: feature-major K, one [128, S] tile per head pair
        # v_aug tiles: token-major V with appended ones column
        k_fm = {}
        v_aug = {}

        def kv_quarter_ops(g):
            """Yield closures, each emitting a small group of tensor ops for
            quarter g's K (head-pairs 2g, 2g+1) and V (heads 4g..4g+3)."""
            for jj in range(2):
                j = 2 * g + jj
                kt_tile = kP.tile([128, S], BF16, tag="kf", name=f"k{j}")
                k_fm[j] = kt_tile
                for c in range(4):
                    def mk(j=j, c=c, kt_tile=kt_tile):
                        pk = psK.tile([128, 512], F32, tag="psk")
                        for d in range(ND):
                            nc.tensor.matmul(
                                pk, wk_all[:, d, j * 128:(j + 1) * 128],
                                zq[c][:, d, :],
                                start=(d == 0), stop=(d == ND - 1))
                        nc.vector.tensor_scalar(
                            kt_tile[:, c * 512:(c + 1) * 512], pk,
                            bk_sb[:, j:j + 1], None, op0=OP.add)
                    yield mk
            for t in range(NT):
                va = vP.tile([128, 4, DK + 1], BF16, tag="va",
                             name=f"v{g}_{t}")
                v_aug[(g, t)] = va

                def mv_(g=g, t=t, va=va):
                    nc.vector.memset(va[:, :, DK:DK + 1], 1.0)
                    pv = psV.tile([128, 256], F32, tag="psv")
                    for d in range(ND):
                        nc.tensor.matmul(
                            pv, zq[t // 4][:, d, (t % 4) * 128:(t % 4 + 1) * 128],
                            wv_all[:, d, g * 256:(g + 1) * 256],
                            start=(d == 0), stop=(d == ND - 1))
                    nc.vector.tensor_add(
                        va[:, :, 0:DK],
                        pv.rearrange("p (h d) -> p h d", h=4),
                        bv_bc[:, g * 256:(g + 1) * 256].rearrange(
                            "p (h d) -> p h d", h=4))
                yield mv_

        # quarter 0 K/V emitted straight (prefix)
        for op in kv_quarter_ops(0):
            op()

        o_fm = [ofmP.tile([128, SQ], BF16, tag=f"o{j}", name=f"o{j}")
                for j in range(ND)]

        # ---- attention: per quarter, with next quarter's K/V as filler ----
        def attention_hp(hp, fillers):
            g = hp // 2
            q2 = 2 * (hp % 2)
            ppv = [ppvp.tile([DK + 1, 512], F32, tag="ppv",
                             name=f"ppv{hp}_{i}") for i in range(2)]
            prev_st = None
            for kt in range(NT + 1):
                if kt < NT:
                    pg = pgp.tile([128, 2, 512], F32, tag="pg")
                    nc.tensor.matmul(
                        pg[:, 0, :],
                        k_fm[hp][0:64, kt * 128:(kt + 1) * 128],
                        q_fm[hp][0:64, :], start=True, stop=True)
                    nc.tensor.matmul(
                        pg[:, 1, :],
                        k_fm[hp][64:128, kt * 128:(kt + 1) * 128],
                        q_fm[hp][64:128, :], start=True, stop=True)
                    stg = stp.tile([128, 2, 512], BF16, tag="st", bufs=4)
                    nc.scalar.activation(stg, pg, AF.Exp, bias=0.0,
                                         scale=0.125)
                if kt > 0:
                    for s in range(2):
                        nc.tensor.matmul(
                            ppv[s],
                            v_aug[(g, kt - 1)][:, q2 + s, :],
                            prev_st[:, s, :],
                            start=(kt == 1), stop=(kt == NT))
                prev_st = stg
                # interleave next-quarter K/V work
                if fillers:
                    fillers.pop(0)()
            # normalization: den -> reciprocal -> broadcast -> scale
            for s in range(2):
                den = stp.tile([1, 512], F32, tag="den", bufs=1)
                nc.vector.tensor_copy(den, ppv[s][DK:DK + 1, :])
                den_r = stp.tile([1, 512], F32, tag="denr", bufs=2)
                nc.vector.reciprocal_approx_fast(den_r, den)
                rb = stp.tile([128, 512], F32, tag="rb", bufs=2)
                nc.gpsimd.partition_broadcast(rb, den_r)
                nc.vector.tensor_copy(o_fm[hp][s * 64:(s + 1) * 64, :],
                                      ppv[s][0:DK, :])
                nc.vector.tensor_mul(o_fm[hp][s * 64:(s + 1) * 64, :],
                                     o_fm[hp][s * 64:(s + 1) * 64, :],
                                     rb[s * 64:(s + 1) * 64, :])

        for g in range(NG):
            fillers = list(kv_quarter_ops(g + 1)) if g + 1 < NG else []
            if g == 3:
                # K/V and z inputs fully consumed; free for output/MLP
                # weights (LIFO: zq on top, then wv, then wk)
                zqStack.close()
                wvP.close()
                wkP.close()
                wo_all[0] = wload(
                    ctx.enter_context(tc.tile_pool(name="woP", bufs=1)),
                    "wo", wot, ND, q=nc.scalar)
                w1_all[0] = wload(
                    ctx.enter_context(tc.tile_pool(name="w1P", bufs=1)),
                    "w1", w1t, ND, q=nc.scalar)
                _w2p = ctx.enter_context(tc.tile_pool(name="w2P", bufs=1))
                w2_all[0] = [None, None]
                for hh in range(2):
                    w2h = walloc(_w2p, f"w2{hh}", ND, D)
                    nc.sync.dma_start(
                        out=w2h,
                        in_=w2t[hh * 1024:(hh + 1) * 1024, :].rearrange(
                            "(d p) c -> p d c", p=128))
                    w2_all[0][hh] = w2h
            attention_hp(2 * g, fillers)
            attention_hp(2 * g + 1, fillers)
            for op in fillers:
                op()

        # ---- out-projection (token-major) + residual + LN2 ----
        # free attention psum pools; psO gets 4 banks, psM 2
        attPs.close()
        mlpCtx = ExitStack()
        z2qP = mlpCtx.enter_context(tc.tile_pool(name="z2qP", bufs=1))
        hP = mlpCtx.enter_context(tc.tile_pool(name="hP", bufs=1))
        psO = mlpCtx.enter_context(tc.tile_pool(name="psO", bufs=2,
                                                space="PSUM"))
        psM = mlpCtx.enter_context(tc.tile_pool(name="psM", bufs=2,
                                                space="PSUM"))
        z2q = z2qP.tile([128, ND, 512], BF16, tag="z2q", name="z2q")
        x_res = [xresP.tile([128, D], BF16, tag=f"xr{t}", name=f"xr{t}")
                 for t in range(NTQ)]

        for t in range(NTQ):
            py = psO.tile([128, 2, 512], F32, tag="pso")
            for u in range(2):
                nc.tensor.matmul(py[:, u, :], ones_sb,
                                 bo_bf[:, u * 512:(u + 1) * 512],
                                 start=True, stop=False)
                for j in range(ND):
                    nc.tensor.matmul(
                        py[:, u, :], o_fm[j][:, t * 128:(t + 1) * 128],
                        wo_all[0][:, j, u * 512:(u + 1) * 512],
                        start=False, stop=(j == ND - 1))
            nc.vector.tensor_add(x_res[t], py.rearrange("p u c -> p (u c)"),
                                 xq[0][:, t, :])
            ln_tile(x_res[t], z2q, t * 128, "z2tm", tq=nc.scalar)

        # ---- MLP ----
        h_fm = [hP.tile([128, SQ], BF16, tag=f"h{f}", name=f"h{f}")
                for f in range(NF)]
        for f in range(NF):
            ph = psM.tile([128, 512], F32, tag="psm")
            for d in range(ND):
                nc.tensor.matmul(ph, w1_all[0][:, d, f * 128:(f + 1) * 128],
                                 z2q[:, d, :], start=(d == 0),
                                 stop=(d == ND - 1))
            nc.scalar.activation(h_fm[f], ph, AF.Relu, bias=b1_sb[:, f:f + 1],
                                 scale=1.0)

        for t in range(NTQ):
            py2 = psO.tile([128, 2, 512], F32, tag="pso")
            for u in range(2):
                nc.tensor.matmul(py2[:, u, :], ones_sb,
                                 b2_bf[:, u * 512:(u + 1) * 512],
                                 start=True, stop=False)
                for f in range(NF):
                    nc.tensor.matmul(
                        py2[:, u, :], h_fm[f][:, t * 128:(t + 1) * 128],
                        w2_all[0][f // 8][:, f % 8, u * 512:(u + 1) * 512],
                        start=False, stop=(f == NF - 1))
            # final residual add in place (x_res fully consumed by LN2)
            nc.vector.tensor_add(x_res[t], py2.rearrange("p u c -> p (u c)"),
                                 x_res[t])
            (nc.sync if t % 2 == 0 else nc.scalar).dma_start(
                out=outd[t * 128:(t + 1) * 128, :], in_=x_res[t])
        mlpCtx.close()

    nc.compile()
    return nc


_LOCK = threading.Lock()
_NC = None


def _get_nc():
    global _NC
    with _LOCK:
        if _NC is None:
            _NC = _build_nc()
    return _NC


def _prep_inputs(inputs):
    x = np.asarray(inputs["x"], np.float32)
    g1 = np.asarray(inputs["ln1_g"], np.float32)
    b1v = np.asarray(inputs["ln1_b"], np.float32)
    g2 = np.asarray(inputs["ln2_g"], np.float32)
    b2v = np.asarray(inputs["ln2_b"], np.float32)
    wq = np.asarray(inputs["wq"], np.float32)
    wk = np.asarray(inputs["wk"], np.float32)
    wv = np.asarray(inputs["wv"], np.float32)
    wo = np.asarray(inputs["wo"], np.float32)
    w1 = np.asarray(inputs["w1"], np.float32)
    w2 = np.asarray(inputs["w2"], np.float32)

    shared = {
        "wqt": np.ascontiguousarray((g1[:, None] * wq.T)).astype(_BF),
        "wkt": np.ascontiguousarray((g1[:, None] * wk.T)).astype(_BF),
        "wvt": np.ascontiguousarray((g1[:, None] * wv.T)).astype(_BF),
        "wot": np.ascontiguousarray(wo.T).astype(_BF),
        "w1t": np.ascontiguousarray((g2[:, None] * w1.T)).astype(_BF),
        "w2t": np.ascontiguousarray(w2.T).astype(_BF),
        "bq": np.ascontiguousarray(
            (inputs["bq"] + wq @ b1v).astype(np.float32).reshape(ND, 128).T),
        "bk": np.ascontiguousarray(
            (inputs["bk"] + wk @ b1v).astype(np.float32).reshape(ND, 128).T),
        "bv": (inputs["bv"] + wv @ b1v).astype(_BF).reshape(1, D),
        "bo": np.asarray(inputs["bo"], _BF).reshape(1, D),
        "b1": np.ascontiguousarray(
            (inputs["b1"] + w1 @ b2v).astype(np.float32).reshape(NF, 128).T),
        "b2": np.asarray(inputs["b2"], _BF).reshape(1, D),
    }

    in_maps = []
    for c in range(NCORES):
        b = c // (NCORES // B)
        qoff = (c % (NCORES // B)) * SQ
        xb = x[b]
        x_perm = np.ascontiguousarray(
            np.concatenate([xb[qoff:qoff + SQ], xb[:qoff], xb[qoff + SQ:]],
                           axis=0)).astype(_BF)
        m = dict(shared)
        m["x"] = x_perm
        in_maps.append(m)
    return in_maps


def _run(inputs, trace=False):
    nc = _get_nc()
    in_maps = _prep_inputs(inputs)
    res = run_bass_kernel_spmd(nc, in_maps, core_ids=list(range(NCORES)),
                               trace=trace)
    out = np.empty((B, S, D), np.float32)
    for c in range(NCORES):
        b = c // (NCORES // B)
        qoff = (c % (NCORES // B)) * SQ
        out[b, qoff:qoff + SQ] = res.results[c]["out"].astype(np.float32)
    return out, res


def kernel(**inputs):
    out, _ = _run(inputs, trace=False)
    return out


# revision 50
# speedup vs baseline: 1.1651x; 1.1651x over previous
"""Trainium2 Bass kernel for a pre-norm transformer decoder layer.

Full inputs in, full output out. 8-way data-parallel over tokens
(batch 2 x 4 query-slices of 512). Each core redundantly computes K/V
for its batch's full 2048-token sequence, interleaved with the
attention loop so the K/V matmuls hide under the ACT-bound softmax exp.

Shapes: x (2, 2048, 1024), 16 heads, dk=64, d_ff=2048, eps=1e-5.
"""
import threading

import numpy as np
import ml_dtypes

import concourse.mybir as mybir
import concourse.tile as tile
from concourse import bacc
from concourse.bass_utils import run_bass_kernel_spmd
from contextlib import ExitStack

F32 = mybir.dt.float32
BF16 = mybir.dt.bfloat16
AF = mybir.ActivationFunctionType
OP = mybir.AluOpType

B, S, D = 2, 2048, 1024
H, DK, FF = 16, 64, 2048
EPS = 1e-5
NCORES = 8
SQ = S * B // NCORES          # 512 own query tokens per core
ND = D // 128                 # 8 feature tiles
NT = S // 128                 # 16 sequence tiles
NTQ = SQ // 128               # 4 own-token tiles
NF = FF // 128                # 16 ff tiles
NHP = H // 2                  # 8 head pairs
NG = 4                        # head-quarter groups (2 head pairs each)

_BF = ml_dtypes.bfloat16


def _build_nc():
    nc = bacc.Bacc("TRN2", target_bir_lowering=False, debug=False,
                   num_devices=NCORES)

    x = nc.dram_tensor("x", [S, D], BF16, kind="ExternalInput").ap()
    wqt = nc.dram_tensor("wqt", [D, D], BF16, kind="ExternalInput").ap()
    wkt = nc.dram_tensor("wkt", [D, D], BF16, kind="ExternalInput").ap()
    wvt = nc.dram_tensor("wvt", [D, D], BF16, kind="ExternalInput").ap()
    wot = nc.dram_tensor("wot", [D, D], BF16, kind="ExternalInput").ap()
    w1t = nc.dram_tensor("w1t", [D, FF], BF16, kind="ExternalInput").ap()
    w2t = nc.dram_tensor("w2t", [FF, D], BF16, kind="ExternalInput").ap()
    bqd = nc.dram_tensor("bq", [128, ND], F32, kind="ExternalInput").ap()
    bkd = nc.dram_tensor("bk", [128, ND], F32, kind="ExternalInput").ap()
    bvd = nc.dram_tensor("bv", [1, D], BF16, kind="ExternalInput").ap()
    bod = nc.dram_tensor("bo", [1, D], BF16, kind="ExternalInput").ap()
    b1d = nc.dram_tensor("b1", [128, NF], F32, kind="ExternalInput").ap()
    b2d = nc.dram_tensor("b2", [1, D], BF16, kind="ExternalInput").ap()
    outd = nc.dram_tensor("out", [SQ, D], BF16, kind="ExternalOutput").ap()

    with tile.TileContext(nc) as tc, ExitStack() as ctx:
        # ---- whole-kernel pools ----
        const = ctx.enter_context(tc.tile_pool(name="const", bufs=1))
        xP = ctx.enter_context(tc.tile_pool(name="xP", bufs=1))
        qfmP = ctx.enter_context(tc.tile_pool(name="qfmP", bufs=1))
        kP = ctx.enter_context(tc.tile_pool(name="kP", bufs=4))
        vP = ctx.enter_context(tc.tile_pool(name="vP", bufs=32))
        ofmP = ctx.enter_context(tc.tile_pool(name="ofmP", bufs=1))
        lns = ctx.enter_context(tc.tile_pool(name="lns", bufs=8))
        stp = ctx.enter_context(tc.tile_pool(name="stp", bufs=3))
        xresP = ctx.enter_context(tc.tile_pool(name="xresP", bufs=1))

        # ---- constants ----
        eps_sb = const.tile([128, 1], F32, tag="eps")
        nc.vector.memset(eps_sb, EPS)
        ones_sb = const.tile([1, 128], BF16, tag="ones")
        nc.vector.memset(ones_sb, 1.0)
        onesc = const.tile([128, 1], BF16, tag="onesc")
        nc.vector.memset(onesc, 1.0)
        bq_sb = const.tile([128, ND], F32, tag="bq")
        bk_sb = const.tile([128, ND], F32, tag="bk")
        bo_bf = const.tile([1, D], BF16, tag="bo")
        b1_sb = const.tile([128, NF], F32, tag="b1")
        b2_bf = const.tile([1, D], BF16, tag="b2")
        bv_bc = const.tile([128, D], BF16, tag="bvb")

        # ---- tile allocation (pool stack: LIFO closes) ----
        def walloc(pool, dname, blocks, cols):
            return pool.tile([128, blocks, cols], BF16, tag=dname, name=dname)

        def wemit(w, dram, q):
            q.dma_start(out=w, in_=dram.rearrange("(d p) c -> p d c", p=128))
            return w

        def wload(pool, dname, dram, blocks, q=nc.gpsimd):
            return wemit(walloc(pool, dname, blocks, dram.shape[1]), dram, q)

        wkP = ExitStack()
        wk_all = walloc(wkP.enter_context(tc.tile_pool(name="wkP", bufs=1)),
                        "wk", ND, D)
        wvP = ExitStack()
        wv_all = walloc(wvP.enter_context(tc.tile_pool(name="wvP", bufs=1)),
                        "wv", ND, D)
        wo_all = [None]   # loaded at g==3, after zq/wk/wv free
        w1_all = [None]
        w2_all = [None]

        # zq[i][p, j, t]: feature-major z quadrant i; zq[i][:, j, t] holds
        # feature j*128+p of token i*512+t
        zqStack = ExitStack()
        zqP = zqStack.enter_context(tc.tile_pool(name="zqP", bufs=1))
        zq = [zqP.tile([128, ND, 512], BF16, tag=f"zq{i}", name=f"zq{i}")
              for i in range(4)]

        wqP = ExitStack()
        wq_all = walloc(wqP.enter_context(tc.tile_pool(name="wqP", bufs=1)),
                        "wq", ND, D)

        # xq[i][p, t, c] = x[i*512 + t*128 + p, c]; quadrant 0 (own tokens)
        # stays resident for the residual adds, 1-3 free after LN1
        xPr = ExitStack()
        xPr_pool = xPr.enter_context(tc.tile_pool(name="xPr", bufs=2))
        xq = [(xP if i == 0 else xPr_pool).tile(
                  [128, 4, D], BF16, tag=("xq0" if i == 0 else "xqr"),
                  name=f"xq{i}") for i in range(4)]

        # ---- DMA emission, bandwidth-prioritized ----
        # Each HW-DGE queue moves ~105 GB/s serially, so split x across the
        # sync and scalar DGEs; weights go through gpsimd's software DGE
        # (that engine is idle until the first attention normalization).
        def xemit(i, q):
            q.dma_start(
                out=xq[i],
                in_=x[i * 512:(i + 1) * 512, :].rearrange(
                    "(t p) c -> p t c", p=128))
        xemit(0, nc.sync)
        xemit(1, nc.scalar)
        nc.sync.dma_start(out=bq_sb, in_=bqd)
        nc.sync.dma_start(out=bk_sb, in_=bkd)
        nc.sync.dma_start(out=bv_bc[0:1, :], in_=bvd)
        nc.sync.dma_start(out=bo_bf, in_=bod)
        nc.sync.dma_start(out=b1_sb, in_=b1d)
        nc.sync.dma_start(out=b2_bf, in_=b2d)
        # wq first on gpsimd's software DGE (idle engine; Q needs it first)
        wemit(wq_all, wqt, nc.gpsimd)
        nc.gpsimd.partition_broadcast(bv_bc, bv_bc[0:1, :])
        wemit(wk_all, wkt, nc.gpsimd)
        wemit(wv_all, wvt, nc.gpsimd)

        # ---- LN1: stats on DVE, apply on ACT, transpose sync/scalar ----

        def ln_tile(x_src, z_dst_q, z_dst_off, tag, tq=None):
            st = lns.tile([128, 2, 6], F32, tag="st")
            nc.vector.bn_stats(st[:, 0, :], x_src[:, 0:512])
            nc.vector.bn_stats(st[:, 1, :], x_src[:, 512:1024])
            mv = lns.tile([128, 2], F32, tag="mv")
            nc.vector.bn_aggr(mv, st)
            sq = lns.tile([128, 1], F32, tag="sq")
            nc.scalar.activation(sq, mv[:, 1:2], AF.Sqrt, bias=eps_sb[:, 0:1],
                                 scale=1.0)
            rstd = lns.tile([128, 1], F32, tag="rstd")
            nc.vector.reciprocal_approx_fast(rstd, sq)
            nmr = lns.tile([128, 1], F32, tag="nmr")
            nc.vector.tensor_scalar(nmr, mv[:, 0:1], rstd, -1.0,
                                    op0=OP.mult, op1=OP.mult)
            z_tm = lns.tile([128, D], BF16, tag=tag, bufs=3)
            nc.scalar.activation(z_tm, x_src, AF.Identity, bias=nmr,
                                 scale=rstd)
            (tq or nc.sync).dma_start_transpose(
                z_dst_q[:, :, z_dst_off:z_dst_off + 128], z_tm)

        # quadrant 0's LN + transposes go ahead of the x2/x3 kicks so the
        # DGEs aren't busy with x when zq[0] (which gates Q) is ready
        for tt in range(4):
            ln_tile(xq[0][:, tt, :], zq[0], tt * 128, "ztm",
                    tq=(nc.sync if tt < 2 else nc.scalar))
        with tc.tile_wait_until(0.022):
            xemit(2, nc.sync)
            xemit(3, nc.scalar)
        for i in range(1, 4):
            for tt in range(4):
                ln_tile(xq[i][:, tt, :], zq[i], tt * 128, "ztm",
                        tq=(nc.sync if tt < 2 else nc.scalar))
        xPr.close()

        # ---- PSUM pools (attention region: 1+1+4+2 = 8 banks) ----
        attPs = ExitStack()
        psK = attPs.enter_context(tc.tile_pool(name="psK", bufs=1,
                                               space="PSUM"))
        psV = attPs.enter_context(tc.tile_pool(name="psV", bufs=1,
                                               space="PSUM"))
        pgp = attPs.enter_context(tc.tile_pool(name="pgp", bufs=2,
                                               space="PSUM"))
        ppvp = attPs.enter_context(tc.tile_pool(name="ppvp", bufs=2,
                                                space="PSUM"))

        # ---- Q: own 512 tokens, feature-major (uses psV's slot early) ----
        q_fm = [qfmP.tile([128, SQ], BF16, tag=f"q{d}", name=f"q{d}")
                for d in range(ND)]
        for j in range(ND):
            pq = pgp.tile([128, 2, 512], F32, tag="pg")
            for d in range(ND):
                nc.tensor.matmul(pq[:, 0, :],
                                 wq_all[:, d, j * 128:(j + 1) * 128],
                                 zq[0][:, d, :],
                                 start=(d == 0), stop=(d == ND - 1))
            nc.vector.tensor_scalar(q_fm[j], pq[:, 0, :], bq_sb[:, j:j + 1],
                                    None, op0=OP.add)
        wqP.close()

        # ---- K/V compute, emitted as "filler" generators per quarter ----
        # k_fm tiles: feature-major K, one [128, S] tile per head pair
        # v_aug tiles: token-major V with appended ones column
        k_fm = {}
        v_aug = {}

        def kv_quarter_ops(g):
            """Yield closures, each emitting a small group of tensor ops for
            quarter g's K (head-pairs 2g, 2g+1) and V (heads 4g..4g+3)."""
            for jj in range(2):
                j = 2 * g + jj
                kt_tile = kP.tile([128, S], BF16, tag="kf", name=f"k{j}")
                k_fm[j] = kt_tile
                for c in range(4):
                    def mk(j=j, c=c, kt_tile=kt_tile):
                        pk = psK.tile([128, 512], F32, tag="psk")
                        for d in range(ND):
                            nc.tensor.matmul(
                                pk, wk_all[:, d, j * 128:(j + 1) * 128],
                                zq[c][:, d, :],
                                start=(d == 0), stop=(d == ND - 1))
                        nc.vector.tensor_scalar(
                            kt_tile[:, c * 512:(c + 1) * 512], pk,
                            bk_sb[:, j:j + 1], None, op0=OP.add)
                    yield mk
            for t in range(NT):
                va = vP.tile([128, 4, DK + 1], BF16, tag="va",
                             name=f"v{g}_{t}")
                v_aug[(g, t)] = va

                def mv_(g=g, t=t, va=va):
                    nc.vector.memset(va[:, :, DK:DK + 1], 1.0)
                    pv = psV.tile([128, 256], F32, tag="psv")
                    for d in range(ND):
                        nc.tensor.matmul(
                            pv, zq[t // 4][:, d, (t % 4) * 128:(t % 4 + 1) * 128],
                            wv_all[:, d, g * 256:(g + 1) * 256],
                            start=(d == 0), stop=(d == ND - 1))
                    nc.vector.tensor_add(
                        va[:, :, 0:DK],
                        pv.rearrange("p (h d) -> p h d", h=4),
                        bv_bc[:, g * 256:(g + 1) * 256].rearrange(
                            "p (h d) -> p h d", h=4))
                yield mv_

        # quarter 0 K/V emitted straight (prefix)
        for op in kv_quarter_ops(0):
            op()

        o_fm = [ofmP.tile([128, SQ], BF16, tag=f"o{j}", name=f"o{j}")
                for j in range(ND)]

        # ---- attention: per quarter, with next quarter's K/V as filler ----
        def attention_hp(hp, fillers):
            g = hp // 2
            q2 = 2 * (hp % 2)
            ppv = [ppvp.tile([DK + 1, 512], F32, tag="ppv",
                             name=f"ppv{hp}_{i}") for i in range(2)]
            prev_st = None
            for kt in range(NT + 1):
                if kt < NT:
                    pg = pgp.tile([128, 2, 512], F32, tag="pg")
                    nc.tensor.matmul(
                        pg[:, 0, :],
                        k_fm[hp][0:64, kt * 128:(kt + 1) * 128],
                        q_fm[hp][0:64, :], start=True, stop=True)
                    nc.tensor.matmul(
                        pg[:, 1, :],
                        k_fm[hp][64:128, kt * 128:(kt + 1) * 128],
                        q_fm[hp][64:128, :], start=True, stop=True)
                    stg = stp.tile([128, 2, 512], BF16, tag="st", bufs=4)
                    nc.scalar.activation(stg, pg, AF.Exp, bias=0.0,
                                         scale=0.125)
                if kt > 0:
                    for s in range(2):
                        nc.tensor.matmul(
                            ppv[s],
                            v_aug[(g, kt - 1)][:, q2 + s, :],
                            prev_st[:, s, :],
                            start=(kt == 1), stop=(kt == NT))
                prev_st = stg
                # interleave next-quarter K/V work
                if fillers:
                    fillers.pop(0)()
            # normalization: den -> reciprocal -> broadcast -> scale
            for s in range(2):
                den = stp.tile([1, 512], F32, tag="den", bufs=1)
                nc.vector.tensor_copy(den, ppv[s][DK:DK + 1, :])
                den_r = stp.tile([1, 512], F32, tag="denr", bufs=2)
                nc.vector.reciprocal_approx_fast(den_r, den)
                rb = stp.tile([128, 512], F32, tag="rb", bufs=2)
                nc.gpsimd.partition_broadcast(rb, den_r)
                nc.vector.tensor_copy(o_fm[hp][s * 64:(s + 1) * 64, :],
                                      ppv[s][0:DK, :])
                nc.vector.tensor_mul(o_fm[hp][s * 64:(s + 1) * 64, :],
                                     o_fm[hp][s * 64:(s + 1) * 64, :],
                                     rb[s * 64:(s + 1) * 64, :])

        for g in range(NG):
            fillers = list(kv_quarter_ops(g + 1)) if g + 1 < NG else []
            if g == 3:
                # K/V and z inputs fully consumed; free for output/MLP
                # weights (LIFO: zq on top, then wv, then wk)
                zqStack.close()
                wvP.close()
                wkP.close()
                wo_all[0] = wload(
                    ctx.enter_context(tc.tile_pool(name="woP", bufs=1)),
                    "wo", wot, ND, q=nc.scalar)
                w1_all[0] = wload(
                    ctx.enter_context(tc.tile_pool(name="w1P", bufs=1)),
                    "w1", w1t, ND, q=nc.scalar)
                _w2p = ctx.enter_context(tc.tile_pool(name="w2P", bufs=1))
                w2_all[0] = [None, None]
                for hh in range(2):
                    w2h = walloc(_w2p, f"w2{hh}", ND, D)
                    nc.sync.dma_start(
                        out=w2h,
                        in_=w2t[hh * 1024:(hh + 1) * 1024, :].rearrange(
                            "(d p) c -> p d c", p=128))
                    w2_all[0][hh] = w2h
            attention_hp(2 * g, fillers)
            attention_hp(2 * g + 1, fillers)
            for op in fillers:
                op()

        # ---- out-projection (token-major) + residual + LN2 ----
        # free attention psum pools; psO gets 4 banks, psM 2
        attPs.close()
        mlpCtx = ExitStack()
        z2qP = mlpCtx.enter_context(tc.tile_pool(name="z2qP", bufs=1))
        hP = mlpCtx.enter_context(tc.tile_pool(name="hP", bufs=1))
        psO = mlpCtx.enter_context(tc.tile_pool(name="psO", bufs=2,
                                                space="PSUM"))
        psM = mlpCtx.enter_context(tc.tile_pool(name="psM", bufs=2,
                                                space="PSUM"))
        z2q = z2qP.tile([128, ND, 512], BF16, tag="z2q", name="z2q")
        x_res = [xresP.tile([128, D], BF16, tag=f"xr{t}", name=f"xr{t}")
                 for t in range(NTQ)]

        for t in range(NTQ):
            py = psO.tile([128, 2, 512], F32, tag="pso")
            for u in range(2):
                nc.tensor.matmul(py[:, u, :], ones_sb,
                                 bo_bf[:, u * 512:(u + 1) * 512],
                                 start=True, stop=False)
                for j in range(ND):
                    nc.tensor.matmul(
                        py[:, u, :], o_fm[j][:, t * 128:(t + 1) * 128],
                        wo_all[0][:, j, u * 512:(u + 1) * 512],
                        start=False, stop=(j == ND - 1))
            nc.vector.tensor_add(x_res[t], py.rearrange("p u c -> p (u c)"),
                                 xq[0][:, t, :])
            ln_tile(x_res[t], z2q, t * 128, "z2tm", tq=nc.scalar)

        # ---- MLP ----
        h_fm = [hP.tile([128, SQ], BF16, tag=f"h{f}", name=f"h{f}")
                for f in range(NF)]
        for f in range(NF):
            ph = psM.tile([128, 512], F32, tag="psm")
            for d in range(ND):
                nc.tensor.matmul(ph, w1_all[0][:, d, f * 128:(f + 1) * 128],
                                 z2q[:, d, :], start=(d == 0),
                                 stop=(d == ND - 1))
            nc.scalar.activation(h_fm[f], ph, AF.Relu, bias=b1_sb[:, f:f + 1],
                                 scale=1.0)

        for t in range(NTQ):
            py2 = psO.tile([128, 2, 512], F32, tag="pso")
            for u in range(2):
                nc.tensor.matmul(py2[:, u, :], ones_sb,
                                 b2_bf[:, u * 512:(u + 1) * 512],
                                 start=True, stop=False)
                for f in range(NF):
                    nc.tensor.matmul(
                        py2[:, u, :], h_fm[f][:, t * 128:(t + 1) * 128],
                        w2_all[0][f // 8][:, f % 8, u * 512:(u + 1) * 512],
                        start=False, stop=(f == NF - 1))
            # final residual add in place (x_res fully consumed by LN2)
            nc.vector.tensor_add(x_res[t], py2.rearrange("p u c -> p (u c)"),
                                 x_res[t])
            (nc.sync if t % 2 == 0 else nc.scalar).dma_start(
                out=outd[t * 128:(t + 1) * 128, :], in_=x_res[t])
        mlpCtx.close()

    nc.compile()
    return nc


_LOCK = threading.Lock()
_NC = None


def _get_nc():
    global _NC
    with _LOCK:
        if _NC is None:
            _NC = _build_nc()
    return _NC


def _prep_inputs(inputs):
    x = np.asarray(inputs["x"], np.float32)
    g1 = np.asarray(inputs["ln1_g"], np.float32)
    b1v = np.asarray(inputs["ln1_b"], np.float32)
    g2 = np.asarray(inputs["ln2_g"], np.float32)
    b2v = np.asarray(inputs["ln2_b"], np.float32)
    wq = np.asarray(inputs["wq"], np.float32)
    wk = np.asarray(inputs["wk"], np.float32)
    wv = np.asarray(inputs["wv"], np.float32)
    wo = np.asarray(inputs["wo"], np.float32)
    w1 = np.asarray(inputs["w1"], np.float32)
    w2 = np.asarray(inputs["w2"], np.float32)

    shared = {
        "wqt": np.ascontiguousarray((g1[:, None] * wq.T)).astype(_BF),
        "wkt": np.ascontiguousarray((g1[:, None] * wk.T)).astype(_BF),
        "wvt": np.ascontiguousarray((g1[:, None] * wv.T)).astype(_BF),
        "wot": np.ascontiguousarray(wo.T).astype(_BF),
        "w1t": np.ascontiguousarray((g2[:, None] * w1.T)).astype(_BF),
        "w2t": np.ascontiguousarray(w2.T).astype(_BF),
        "bq": np.ascontiguousarray(
            (inputs["bq"] + wq @ b1v).astype(np.float32).reshape(ND, 128).T),
        "bk": np.ascontiguousarray(
            (inputs["bk"] + wk @ b1v).astype(np.float32).reshape(ND, 128).T),
        "bv": (inputs["bv"] + wv @ b1v).astype(_BF).reshape(1, D),
        "bo": np.asarray(inputs["bo"], _BF).reshape(1, D),
        "b1": np.ascontiguousarray(
            (inputs["b1"] + w1 @ b2v).astype(np.float32).reshape(NF, 128).T),
        "b2": np.asarray(inputs["b2"], _BF).reshape(1, D),
    }

    in_maps = []
    for c in range(NCORES):
        b = c // (NCORES // B)
        qoff = (c % (NCORES // B)) * SQ
        xb = x[b]
        x_perm = np.ascontiguousarray(
            np.concatenate([xb[qoff:qoff + SQ], xb[:qoff], xb[qoff + SQ:]],
                           axis=0)).astype(_BF)
        m = dict(shared)
        m["x"] = x_perm
        in_maps.append(m)
    return in_maps


def _run(inputs, trace=False):
    nc = _get_nc()
    in_maps = _prep_inputs(inputs)
    res = run_bass_kernel_spmd(nc, in_maps, core_ids=list(range(NCORES)),
                               trace=trace)
    out = np.empty((B, S, D), np.float32)
    for c in range(NCORES):
        b = c // (NCORES // B)
        qoff = (c % (NCORES // B)) * SQ
        out[b, qoff:qoff + SQ] = res.results[c]["out"].astype(np.float32)
    return out, res


def kernel(**inputs):
    out, _ = _run(inputs, trace=False)
    return out


# revision 51
# speedup vs baseline: 1.1690x; 1.0034x over previous
"""Trainium2 Bass kernel for a pre-norm transformer decoder layer.

Full inputs in, full output out. 8-way data-parallel over tokens
(batch 2 x 4 query-slices of 512). Each core redundantly computes K/V
for its batch's full 2048-token sequence, interleaved with the
attention loop so the K/V matmuls hide under the ACT-bound softmax exp.

Shapes: x (2, 2048, 1024), 16 heads, dk=64, d_ff=2048, eps=1e-5.
"""
import threading

import numpy as np
import ml_dtypes

import concourse.mybir as mybir
import concourse.tile as tile
from concourse import bacc
from concourse.bass_utils import run_bass_kernel_spmd
from contextlib import ExitStack

F32 = mybir.dt.float32
BF16 = mybir.dt.bfloat16
AF = mybir.ActivationFunctionType
OP = mybir.AluOpType

B, S, D = 2, 2048, 1024
H, DK, FF = 16, 64, 2048
EPS = 1e-5
NCORES = 8
SQ = S * B // NCORES          # 512 own query tokens per core
ND = D // 128                 # 8 feature tiles
NT = S // 128                 # 16 sequence tiles
NTQ = SQ // 128               # 4 own-token tiles
NF = FF // 128                # 16 ff tiles
NHP = H // 2                  # 8 head pairs
NG = 4                        # head-quarter groups (2 head pairs each)

_BF = ml_dtypes.bfloat16


def _build_nc():
    nc = bacc.Bacc("TRN2", target_bir_lowering=False, debug=False,
                   num_devices=NCORES)

    x = nc.dram_tensor("x", [S, D], BF16, kind="ExternalInput").ap()
    wqt = nc.dram_tensor("wqt", [D, D], BF16, kind="ExternalInput").ap()
    wkt = nc.dram_tensor("wkt", [D, D], BF16, kind="ExternalInput").ap()
    wvt = nc.dram_tensor("wvt", [D, D], BF16, kind="ExternalInput").ap()
    wot = nc.dram_tensor("wot", [D, D], BF16, kind="ExternalInput").ap()
    w1t = nc.dram_tensor("w1t", [D, FF], BF16, kind="ExternalInput").ap()
    w2t = nc.dram_tensor("w2t", [FF, D], BF16, kind="ExternalInput").ap()
    bqd = nc.dram_tensor("bq", [128, ND], F32, kind="ExternalInput").ap()
    bkd = nc.dram_tensor("bk", [128, ND], F32, kind="ExternalInput").ap()
    bvd = nc.dram_tensor("bv", [1, D], BF16, kind="ExternalInput").ap()
    bod = nc.dram_tensor("bo", [1, D], BF16, kind="ExternalInput").ap()
    b1d = nc.dram_tensor("b1", [128, NF], F32, kind="ExternalInput").ap()
    b2d = nc.dram_tensor("b2", [1, D], BF16, kind="ExternalInput").ap()
    outd = nc.dram_tensor("out", [SQ, D], BF16, kind="ExternalOutput").ap()

    with tile.TileContext(nc) as tc, ExitStack() as ctx:
        # ---- whole-kernel pools ----
        const = ctx.enter_context(tc.tile_pool(name="const", bufs=1))
        xP = ctx.enter_context(tc.tile_pool(name="xP", bufs=1))
        qfmP = ctx.enter_context(tc.tile_pool(name="qfmP", bufs=1))
        kP = ctx.enter_context(tc.tile_pool(name="kP", bufs=4))
        vP = ctx.enter_context(tc.tile_pool(name="vP", bufs=32))
        ofmP = ctx.enter_context(tc.tile_pool(name="ofmP", bufs=1))
        lns = ctx.enter_context(tc.tile_pool(name="lns", bufs=8))
        stp = ctx.enter_context(tc.tile_pool(name="stp", bufs=3))
        xresP = ctx.enter_context(tc.tile_pool(name="xresP", bufs=1))

        # ---- constants ----
        eps_sb = const.tile([128, 1], F32, tag="eps")
        nc.vector.memset(eps_sb, EPS)
        ones_sb = const.tile([1, 128], BF16, tag="ones")
        nc.vector.memset(ones_sb, 1.0)
        onesc = const.tile([128, 1], BF16, tag="onesc")
        nc.vector.memset(onesc, 1.0)
        bq_sb = const.tile([128, ND], F32, tag="bq")
        bk_sb = const.tile([128, ND], F32, tag="bk")
        bo_bf = const.tile([1, D], BF16, tag="bo")
        b1_sb = const.tile([128, NF], F32, tag="b1")
        b2_bf = const.tile([1, D], BF16, tag="b2")
        bv_bc = const.tile([128, D], BF16, tag="bvb")

        # ---- tile allocation (pool stack: LIFO closes) ----
        def walloc(pool, dname, blocks, cols):
            return pool.tile([128, blocks, cols], BF16, tag=dname, name=dname)

        def wemit(w, dram, q):
            q.dma_start(out=w, in_=dram.rearrange("(d p) c -> p d c", p=128))
            return w

        def wload(pool, dname, dram, blocks, q=nc.gpsimd):
            return wemit(walloc(pool, dname, blocks, dram.shape[1]), dram, q)

        wkP = ExitStack()
        wk_all = walloc(wkP.enter_context(tc.tile_pool(name="wkP", bufs=1)),
                        "wk", ND, D)
        wvP = ExitStack()
        wv_all = walloc(wvP.enter_context(tc.tile_pool(name="wvP", bufs=1)),
                        "wv", ND, D)
        wo_all = [None]   # loaded at g==3, after zq/wk/wv free
        w1_all = [None]
        w2_all = [None]

        # zq[i][p, j, t]: feature-major z quadrant i; zq[i][:, j, t] holds
        # feature j*128+p of token i*512+t
        zqStack = ExitStack()
        zqP = zqStack.enter_context(tc.tile_pool(name="zqP", bufs=1))
        zq = [zqP.tile([128, ND, 512], BF16, tag=f"zq{i}", name=f"zq{i}")
              for i in range(4)]

        wqP = ExitStack()
        wq_all = walloc(wqP.enter_context(tc.tile_pool(name="wqP", bufs=1)),
                        "wq", ND, D)

        # xq[i][p, t, c] = x[i*512 + t*128 + p, c]; quadrant 0 (own tokens)
        # stays resident for the residual adds, 1-3 free after LN1
        xPr = ExitStack()
        xPr_pool = xPr.enter_context(tc.tile_pool(name="xPr", bufs=2))
        xq = [(xP if i == 0 else xPr_pool).tile(
                  [128, 4, D], BF16, tag=("xq0" if i == 0 else "xqr"),
                  name=f"xq{i}") for i in range(4)]

        # ---- DMA emission, bandwidth-prioritized ----
        # Each HW-DGE queue moves ~105 GB/s serially, so split x across the
        # sync and scalar DGEs; weights go through gpsimd's software DGE
        # (that engine is idle until the first attention normalization).
        def xemit(i, q):
            q.dma_start(
                out=xq[i],
                in_=x[i * 512:(i + 1) * 512, :].rearrange(
                    "(t p) c -> p t c", p=128))
        xemit(0, nc.sync)
        xemit(1, nc.scalar)
        nc.sync.dma_start(out=bq_sb, in_=bqd)
        nc.sync.dma_start(out=bk_sb, in_=bkd)
        nc.sync.dma_start(out=bv_bc[0:1, :], in_=bvd)
        nc.sync.dma_start(out=bo_bf, in_=bod)
        nc.sync.dma_start(out=b1_sb, in_=b1d)
        nc.sync.dma_start(out=b2_bf, in_=b2d)
        # wq first on gpsimd's software DGE (idle engine; Q needs it first)
        wemit(wq_all, wqt, nc.gpsimd)
        nc.gpsimd.partition_broadcast(bv_bc, bv_bc[0:1, :])
        wemit(wk_all, wkt, nc.gpsimd)
        wemit(wv_all, wvt, nc.gpsimd)

        # ---- LN1: stats on DVE, apply on ACT, transpose sync/scalar ----

        def ln_tile(x_src, z_dst_q, z_dst_off, tag, tq=None):
            st = lns.tile([128, 2, 6], F32, tag="st")
            nc.vector.bn_stats(st[:, 0, :], x_src[:, 0:512])
            nc.vector.bn_stats(st[:, 1, :], x_src[:, 512:1024])
            mv = lns.tile([128, 2], F32, tag="mv")
            nc.vector.bn_aggr(mv, st)
            sq = lns.tile([128, 1], F32, tag="sq")
            nc.scalar.activation(sq, mv[:, 1:2], AF.Sqrt, bias=eps_sb[:, 0:1],
                                 scale=1.0)
            rstd = lns.tile([128, 1], F32, tag="rstd")
            nc.vector.reciprocal_approx_fast(rstd, sq)
            nmr = lns.tile([128, 1], F32, tag="nmr")
            nc.vector.tensor_scalar(nmr, mv[:, 0:1], rstd, -1.0,
                                    op0=OP.mult, op1=OP.mult)
            z_tm = lns.tile([128, D], BF16, tag=tag, bufs=3)
            nc.scalar.activation(z_tm, x_src, AF.Identity, bias=nmr,
                                 scale=rstd)
            (tq or nc.sync).dma_start_transpose(
                z_dst_q[:, :, z_dst_off:z_dst_off + 128], z_tm)

        # quadrant 0's LN + transposes go ahead of the x2/x3 kicks so the
        # DGEs aren't busy with x when zq[0] (which gates Q) is ready
        for tt in range(4):
            ln_tile(xq[0][:, tt, :], zq[0], tt * 128, "ztm",
                    tq=(nc.sync if tt < 2 else nc.scalar))
        with tc.tile_wait_until(0.022):
            xemit(2, nc.sync)
            xemit(3, nc.sync)
        for i in range(1, 4):
            for tt in range(4):
                ln_tile(xq[i][:, tt, :], zq[i], tt * 128, "ztm",
                        tq=(nc.sync if tt < 2 else nc.scalar))
        xPr.close()

        # ---- PSUM pools (attention region: 1+1+4+2 = 8 banks) ----
        attPs = ExitStack()
        psK = attPs.enter_context(tc.tile_pool(name="psK", bufs=1,
                                               space="PSUM"))
        psV = attPs.enter_context(tc.tile_pool(name="psV", bufs=1,
                                               space="PSUM"))
        pgp = attPs.enter_context(tc.tile_pool(name="pgp", bufs=2,
                                               space="PSUM"))
        ppvp = attPs.enter_context(tc.tile_pool(name="ppvp", bufs=2,
                                                space="PSUM"))

        # ---- Q: own 512 tokens, feature-major (uses psV's slot early) ----
        q_fm = [qfmP.tile([128, SQ], BF16, tag=f"q{d}", name=f"q{d}")
                for d in range(ND)]
        for j in range(ND):
            pq = pgp.tile([128, 2, 512], F32, tag="pg")
            for d in range(ND):
                nc.tensor.matmul(pq[:, 0, :],
                                 wq_all[:, d, j * 128:(j + 1) * 128],
                                 zq[0][:, d, :],
                                 start=(d == 0), stop=(d == ND - 1))
            nc.vector.tensor_scalar(q_fm[j], pq[:, 0, :], bq_sb[:, j:j + 1],
                                    None, op0=OP.add)
        wqP.close()

        # ---- K/V compute, emitted as "filler" generators per quarter ----
        # k_fm tiles: feature-major K, one [128, S] tile per head pair
        # v_aug tiles: token-major V with appended ones column
        k_fm = {}
        v_aug = {}

        def kv_quarter_ops(g):
            """Yield closures, each emitting a small group of tensor ops for
            quarter g's K (head-pairs 2g, 2g+1) and V (heads 4g..4g+3)."""
            for jj in range(2):
                j = 2 * g + jj
                kt_tile = kP.tile([128, S], BF16, tag="kf", name=f"k{j}")
                k_fm[j] = kt_tile
                for c in range(4):
                    def mk(j=j, c=c, kt_tile=kt_tile):
                        pk = psK.tile([128, 512], F32, tag="psk")
                        for d in range(ND):
                            nc.tensor.matmul(
                                pk, wk_all[:, d, j * 128:(j + 1) * 128],
                                zq[c][:, d, :],
                                start=(d == 0), stop=(d == ND - 1))
                        nc.vector.tensor_scalar(
                            kt_tile[:, c * 512:(c + 1) * 512], pk,
                            bk_sb[:, j:j + 1], None, op0=OP.add)
                    yield mk
            for t in range(NT):
                va = vP.tile([128, 4, DK + 1], BF16, tag="va",
                             name=f"v{g}_{t}")
                v_aug[(g, t)] = va

                def mv_(g=g, t=t, va=va):
                    nc.vector.memset(va[:, :, DK:DK + 1], 1.0)
                    pv = psV.tile([128, 256], F32, tag="psv")
                    for d in range(ND):
                        nc.tensor.matmul(
                            pv, zq[t // 4][:, d, (t % 4) * 128:(t % 4 + 1) * 128],
                            wv_all[:, d, g * 256:(g + 1) * 256],
                            start=(d == 0), stop=(d == ND - 1))
                    nc.vector.tensor_add(
                        va[:, :, 0:DK],
                        pv.rearrange("p (h d) -> p h d", h=4),
                        bv_bc[:, g * 256:(g + 1) * 256].rearrange(
                            "p (h d) -> p h d", h=4))
                yield mv_

        # quarter 0 K/V emitted straight (prefix)
        for op in kv_quarter_ops(0):
            op()

        o_fm = [ofmP.tile([128, SQ], BF16, tag=f"o{j}", name=f"o{j}")
                for j in range(ND)]

        # ---- attention: per quarter, with next quarter's K/V as filler ----
        def attention_hp(hp, fillers):
            g = hp // 2
            q2 = 2 * (hp % 2)
            ppv = [ppvp.tile([DK + 1, 512], F32, tag="ppv",
                             name=f"ppv{hp}_{i}") for i in range(2)]
            prev_st = None
            for kt in range(NT + 1):
                if kt < NT:
                    pg = pgp.tile([128, 2, 512], F32, tag="pg")
                    nc.tensor.matmul(
                        pg[:, 0, :],
                        k_fm[hp][0:64, kt * 128:(kt + 1) * 128],
                        q_fm[hp][0:64, :], start=True, stop=True)
                    nc.tensor.matmul(
                        pg[:, 1, :],
                        k_fm[hp][64:128, kt * 128:(kt + 1) * 128],
                        q_fm[hp][64:128, :], start=True, stop=True)
                    stg = stp.tile([128, 2, 512], BF16, tag="st", bufs=4)
                    nc.scalar.activation(stg, pg, AF.Exp, bias=0.0,
                                         scale=0.125)
                if kt > 0:
                    for s in range(2):
                        nc.tensor.matmul(
                            ppv[s],
                            v_aug[(g, kt - 1)][:, q2 + s, :],
                            prev_st[:, s, :],
                            start=(kt == 1), stop=(kt == NT))
                prev_st = stg
                # interleave next-quarter K/V work
                if fillers:
                    fillers.pop(0)()
            # normalization: den -> reciprocal -> broadcast -> scale
            for s in range(2):
                den = stp.tile([1, 512], F32, tag="den", bufs=1)
                nc.vector.tensor_copy(den, ppv[s][DK:DK + 1, :])
                den_r = stp.tile([1, 512], F32, tag="denr", bufs=2)
                nc.vector.reciprocal_approx_fast(den_r, den)
                rb = stp.tile([128, 512], F32, tag="rb", bufs=2)
                nc.gpsimd.partition_broadcast(rb, den_r)
                nc.vector.tensor_copy(o_fm[hp][s * 64:(s + 1) * 64, :],
                                      ppv[s][0:DK, :])
                nc.vector.tensor_mul(o_fm[hp][s * 64:(s + 1) * 64, :],
                                     o_fm[hp][s * 64:(s + 1) * 64, :],
                                     rb[s * 64:(s + 1) * 64, :])

        for g in range(NG):
            fillers = list(kv_quarter_ops(g + 1)) if g + 1 < NG else []
            if g == 3:
                # K/V and z inputs fully consumed; free for output/MLP
                # weights (LIFO: zq on top, then wv, then wk)
                zqStack.close()
                wvP.close()
                wkP.close()
                wo_all[0] = wload(
                    ctx.enter_context(tc.tile_pool(name="woP", bufs=1)),
                    "wo", wot, ND, q=nc.scalar)
                w1_all[0] = wload(
                    ctx.enter_context(tc.tile_pool(name="w1P", bufs=1)),
                    "w1", w1t, ND, q=nc.scalar)
                _w2p = ctx.enter_context(tc.tile_pool(name="w2P", bufs=1))
                w2_all[0] = [None, None]
                for hh in range(2):
                    w2h = walloc(_w2p, f"w2{hh}", ND, D)
                    nc.sync.dma_start(
                        out=w2h,
                        in_=w2t[hh * 1024:(hh + 1) * 1024, :].rearrange(
                            "(d p) c -> p d c", p=128))
                    w2_all[0][hh] = w2h
            attention_hp(2 * g, fillers)
            attention_hp(2 * g + 1, fillers)
            for op in fillers:
                op()

        # ---- out-projection (token-major) + residual + LN2 ----
        # free attention psum pools; psO gets 4 banks, psM 2
        attPs.close()
        mlpCtx = ExitStack()
        z2qP = mlpCtx.enter_context(tc.tile_pool(name="z2qP", bufs=1))
        hP = mlpCtx.enter_context(tc.tile_pool(name="hP", bufs=1))
        psO = mlpCtx.enter_context(tc.tile_pool(name="psO", bufs=2,
                                                space="PSUM"))
        psM = mlpCtx.enter_context(tc.tile_pool(name="psM", bufs=2,
                                                space="PSUM"))
        z2q = z2qP.tile([128, ND, 512], BF16, tag="z2q", name="z2q")
        x_res = [xresP.tile([128, D], BF16, tag=f"xr{t}", name=f"xr{t}")
                 for t in range(NTQ)]

        for t in range(NTQ):
            py = psO.tile([128, 2, 512], F32, tag="pso")
            for u in range(2):
                nc.tensor.matmul(py[:, u, :], ones_sb,
                                 bo_bf[:, u * 512:(u + 1) * 512],
                                 start=True, stop=False)
                for j in range(ND):
                    nc.tensor.matmul(
                        py[:, u, :], o_fm[j][:, t * 128:(t + 1) * 128],
                        wo_all[0][:, j, u * 512:(u + 1) * 512],
                        start=False, stop=(j == ND - 1))
            nc.vector.tensor_add(x_res[t], py.rearrange("p u c -> p (u c)"),
                                 xq[0][:, t, :])
            ln_tile(x_res[t], z2q, t * 128, "z2tm", tq=nc.scalar)

        # ---- MLP ----
        h_fm = [hP.tile([128, SQ], BF16, tag=f"h{f}", name=f"h{f}")
                for f in range(NF)]
        for f in range(NF):
            ph = psM.tile([128, 512], F32, tag="psm")
            for d in range(ND):
                nc.tensor.matmul(ph, w1_all[0][:, d, f * 128:(f + 1) * 128],
                                 z2q[:, d, :], start=(d == 0),
                                 stop=(d == ND - 1))
            nc.scalar.activation(h_fm[f], ph, AF.Relu, bias=b1_sb[:, f:f + 1],
                                 scale=1.0)

        for t in range(NTQ):
            py2 = psO.tile([128, 2, 512], F32, tag="pso")
            for u in range(2):
                nc.tensor.matmul(py2[:, u, :], ones_sb,
                                 b2_bf[:, u * 512:(u + 1) * 512],
                                 start=True, stop=False)
                for f in range(NF):
                    nc.tensor.matmul(
                        py2[:, u, :], h_fm[f][:, t * 128:(t + 1) * 128],
                        w2_all[0][f // 8][:, f % 8, u * 512:(u + 1) * 512],
                        start=False, stop=(f == NF - 1))
            # final residual add in place (x_res fully consumed by LN2)
            nc.vector.tensor_add(x_res[t], py2.rearrange("p u c -> p (u c)"),
                                 x_res[t])
            (nc.sync if t % 2 == 0 else nc.scalar).dma_start(
                out=outd[t * 128:(t + 1) * 128, :], in_=x_res[t])
        mlpCtx.close()

    nc.compile()
    return nc


_LOCK = threading.Lock()
_NC = None


def _get_nc():
    global _NC
    with _LOCK:
        if _NC is None:
            _NC = _build_nc()
    return _NC


def _prep_inputs(inputs):
    x = np.asarray(inputs["x"], np.float32)
    g1 = np.asarray(inputs["ln1_g"], np.float32)
    b1v = np.asarray(inputs["ln1_b"], np.float32)
    g2 = np.asarray(inputs["ln2_g"], np.float32)
    b2v = np.asarray(inputs["ln2_b"], np.float32)
    wq = np.asarray(inputs["wq"], np.float32)
    wk = np.asarray(inputs["wk"], np.float32)
    wv = np.asarray(inputs["wv"], np.float32)
    wo = np.asarray(inputs["wo"], np.float32)
    w1 = np.asarray(inputs["w1"], np.float32)
    w2 = np.asarray(inputs["w2"], np.float32)

    shared = {
        "wqt": np.ascontiguousarray((g1[:, None] * wq.T)).astype(_BF),
        "wkt": np.ascontiguousarray((g1[:, None] * wk.T)).astype(_BF),
        "wvt": np.ascontiguousarray((g1[:, None] * wv.T)).astype(_BF),
        "wot": np.ascontiguousarray(wo.T).astype(_BF),
        "w1t": np.ascontiguousarray((g2[:, None] * w1.T)).astype(_BF),
        "w2t": np.ascontiguousarray(w2.T).astype(_BF),
        "bq": np.ascontiguousarray(
            (inputs["bq"] + wq @ b1v).astype(np.float32).reshape(ND, 128).T),
        "bk": np.ascontiguousarray(
            (inputs["bk"] + wk @ b1v).astype(np.float32).reshape(ND, 128).T),
        "bv": (inputs["bv"] + wv @ b1v).astype(_BF).reshape(1, D),
        "bo": np.asarray(inputs["bo"], _BF).reshape(1, D),
        "b1": np.ascontiguousarray(
            (inputs["b1"] + w1 @ b2v).astype(np.float32).reshape(NF, 128).T),
        "b2": np.asarray(inputs["b2"], _BF).reshape(1, D),
    }

    in_maps = []
    for c in range(NCORES):
        b = c // (NCORES // B)
        qoff = (c % (NCORES // B)) * SQ
        xb = x[b]
        x_perm = np.ascontiguousarray(
            np.concatenate([xb[qoff:qoff + SQ], xb[:qoff], xb[qoff + SQ:]],
                           axis=0)).astype(_BF)
        m = dict(shared)
        m["x"] = x_perm
        in_maps.append(m)
    return in_maps


def _run(inputs, trace=False):
    nc = _get_nc()
    in_maps = _prep_inputs(inputs)
    res = run_bass_kernel_spmd(nc, in_maps, core_ids=list(range(NCORES)),
                               trace=trace)
    out = np.empty((B, S, D), np.float32)
    for c in range(NCORES):
        b = c // (NCORES // B)
        qoff = (c % (NCORES // B)) * SQ
        out[b, qoff:qoff + SQ] = res.results[c]["out"].astype(np.float32)
    return out, res


def kernel(**inputs):
    out, _ = _run(inputs, trace=False)
    return out
